# revision 35
# baseline (speedup 1.0000x reference)
"""Self-contained Trainium2 Bass kernel for causal multi-head attention.

Problem: B=2, S=2048, D=1024, H=16 heads (dk=64), fp32, causal + padding mask.
Sharding across 8 NeuronCores: core c -> batch c//4, head-group c%4 (4 heads).

Device-side design (v4 — reversed phase order):
  - All matmul operands bf16 (cast on host); fp32 PSUM accumulation.
  - qT/kT transposed [dk, S]; scores S_T[k, q] with the head pair
    row-packed into PE rows 0-63 / 64-127: the two K=64 matmuls issue
    back-to-back and co-execute on disjoint row groups (measured 0-25ns
    issue deltas), so a score block pair costs ~512 cols, not 1024.
  - Padding via the exp's per-partition bias (-1e9 on padded keys).
  - Causal: DVE multiply of the exp'd diagonal 128-block by a 0/1
    triangle (replaces the v3 PE triangle-add matmuls: the exp of an
    unmasked score is finite, then zeroed, so PV and the denominator
    stay exact).
  - q bias via DVE tensor_scalar_add during the PSUM->SBUF copy
    (replaces the v3 K=1 bias matmuls). k bias provably cancels in
    softmax; v/o biases fold on host.
  - Query chunks processed HIGH to LOW (qc3..qc0). The exp work per
    chunk is ~2.5x its PE score+PV work, so late chunks (16 kb blocks)
    would starve the PE and trip the HAM 50% activity throttle at the
    tail; reversed, qc3's exp backlog overlaps the Q/V projections and
    the tail phase is the 4-block qc0 with out-projections as filler.
  - Deferred PE work (Q-proj of chunks 0-2, out-projections lagging one
    phase) is interleaved between attention blocks at ~1-3us grain so
    the PE queue never blocks on the ACT exp backlog.
  - Normalization (unchanged from v3): ctx^T+den in PSUM rows 0-64,
    copied once to SBUF; reciprocal via DVE; DMA broadcast through a
    DRAM bounce; multiplies deferred one half-phase. Tail fast path
    keeps the last heads' denominator on one partition.
Fully-masked rows are overwritten on host with the uniform-attention
reference value.
"""

import numpy as np
from contextlib import ExitStack

import concourse.bass as bass
import concourse.bacc as bacc
import concourse.tile as tile
import concourse.mybir as mybir
from concourse.bass import ds, ts

F32 = mybir.dt.float32
FR = mybir.dt.float32r
BF = mybir.dt.bfloat16
AF = mybir.ActivationFunctionType

P = 128
S = 2048
D = 1024
HL = 4          # heads per core
DK = 64
KT = D // P     # 8 k-tiles over the model dim
ST = S // P     # 16 seq tiles
NQC = 4         # 512-wide query chunks
PADNEG = -1.0e9  # post-scale padding bias; exp(-1e9) = 0
N_CORES = 8
N_HEAD = 16


def build_program(num_devices=N_CORES):
    nc = bacc.Bacc(
        "TRN2",
        target_bir_lowering=False,
        debug=False,
        enable_asserts=True,
        num_devices=num_devices,
    )
    ins = {
        # all bulk inputs pre-swizzled on host to partition-major layouts so
        # each arrives in 1-2 large contiguous DMA descriptors (the input
        # stream is descriptor-issue-bound, ~600ns engine time per dma_start)
        "xt": nc.dram_tensor("xt", [P, NQC, KT, 512], BF, kind="ExternalInput").ap(),
        "wq": nc.dram_tensor("wq", [P, KT, 2 * P], BF, kind="ExternalInput").ap(),
        "wk": nc.dram_tensor("wk", [P, KT, 2 * P], BF, kind="ExternalInput").ap(),
        "wv": nc.dram_tensor("wv", [P, KT, 2 * P], BF, kind="ExternalInput").ap(),
        "wo": nc.dram_tensor("wo", [2 * P, D], BF, kind="ExternalInput").ap(),
        "bq2": nc.dram_tensor("bq2", [P, 2], F32, kind="ExternalInput").ap(),
        "padb": nc.dram_tensor("padb", [P, ST], F32, kind="ExternalInput").ap(),
        "tri2": nc.dram_tensor("tri2", [P, 2 * P], BF, kind="ExternalInput").ap(),
    }
    y = nc.dram_tensor("y", [S, D], BF, kind="ExternalOutput").ap()
    ins["rcp_dram"] = nc.dram_tensor("rcp_dram", [2 * NQC * 2, 512], F32).ap()

    with tile.TileContext(nc) as tc:
        _body(tc, y, ins)

    nc.compile()
    return nc


def _body(tc, y, ins):
    nc = tc.nc

    with ExitStack() as ctx:
        const = ctx.enter_context(tc.tile_pool(name="const", bufs=1))
        pt_pool = ctx.enter_context(tc.tile_pool(name="pt", bufs=4))
        rrp = ctx.enter_context(tc.tile_pool(name="rr", bufs=2))
        ysb = ctx.enter_context(tc.tile_pool(name="ysb", bufs=4))
        psA = ctx.enter_context(tc.tile_pool(name="psA", bufs=2, space="PSUM"))
        psB = ctx.enter_context(tc.tile_pool(name="psB", bufs=2, space="PSUM"))
        psY = ctx.enter_context(tc.tile_pool(name="psY", bufs=2, space="PSUM"))

        ones_bf = const.tile([1, 512], BF)
        nc.vector.memset(ones_bf[:], 1.0)

        # ---------------- input DMAs ----------------
        # priority order, emissions alternating across the two HWDGE
        # queues: tiny tensors, wk, xt chunk3 (qc3 needs kt3+qt3 first),
        # wq, wv, xt chunks 0,1,2, wo last (needed only at outproj(3)).
        bq2_sb = const.tile([P, 2], F32)
        nc.sync.dma_start(bq2_sb[:], ins["bq2"])
        padb_sb = const.tile([P, ST], F32)
        nc.sync.dma_start(padb_sb[:], ins["padb"])
        tri2_sb = const.tile([P, 2 * P], BF)
        nc.sync.dma_start(tri2_sb[:], ins["tri2"])

        xt_sb = const.tile([P, NQC, KT, 512], BF)
        wq_sb = const.tile([P, KT, 2 * P], BF)
        wk_sb = const.tile([P, KT, 2 * P], BF)
        wv_sb = const.tile([P, KT, 2 * P], BF)

        # each tensor split k-wise in halves across the two HWDGE queues;
        # priority: wk, xt chunk3 (qc3 runs first), wq, wv, xt chunks 0,1,2
        def half_dma(dst, src):
            nc.sync.dma_start(dst[:, 0 : KT // 2], src[:, 0 : KT // 2])
            nc.scalar.dma_start(dst[:, KT // 2 :], src[:, KT // 2 :])

        half_dma(wk_sb, ins["wk"])
        half_dma(xt_sb[:, 0], ins["xt"][:, 0])
        half_dma(wq_sb, ins["wq"])
        half_dma(xt_sb[:, 3], ins["xt"][:, 3])
        half_dma(wv_sb, ins["wv"])
        for n in (1, 2):
            half_dma(xt_sb[:, n], ins["xt"][:, n])

        # wo: bf16, head pair m stacked as 128 contraction rows
        wo_sb = const.tile([P, 2, D], BF)
        wo_r = ins["wo"].rearrange("(m p) n -> m p n", p=P)
        for m in range(2):
            nc.scalar.dma_start(wo_sb[:, m], wo_r[m])
        # head 3's rows again at partitions 0-63: the tail out-projection
        # consumes head 3's ctx in place (no partition shift on the tail)
        wo3_sb = const.tile([DK, D], BF)
        nc.scalar.dma_start(wo3_sb[:], wo_r[1][DK:P, :])

        qt_sb = const.tile([P, 2, S], BF)
        kt_sb = const.tile([P, 2, S], BF)
        # per head: 64 value cols + ones denominator col; padded so a
        # 128-wide stationary slice starting at h*65 stays in bounds (the
        # extra columns produce junk output rows 65-127, never read)
        VW = HL * (DK + 1) + DK - 1  # 323
        vaug_sb = const.tile([P, ST, VW], BF)
        den_cols = vaug_sb[:, :, 0 : HL * (DK + 1)].rearrange(
            "p s (h c) -> p s h c", c=DK + 1
        )[:, :, :, DK : DK + 1]
        nc.vector.memset(den_cols, 1.0)
        nc.vector.memset(vaug_sb[:, :, HL * (DK + 1) : VW], 0.0)

        # normalized context head pairs [128, 512] bf16: rows 0-63 head 2m,
        # rows 64-127 head 2m+1 (DMA-shifted in)
        ctx_sets = []
        for st in range(2):
            tiles = []
            for m in range(2):
                t = const.tile([P, 512], BF, name=f"ctxsb{st}_{m}", tag=f"ctxsb{st}_{m}")
                tiles.append(t)
            ctx_sets.append(tiles)

        # ---------------- projections (m-granular, psY ring) ----------------
        def proj_qk(n, m, tgt, w_sb, bias):
            """One [128,512] chunk of the q or k projection (chunk n, half m)."""
            ps = psY.tile([P, 512], F32, name=f"ps_{('k','q')[bias]}{n}_{m}", tag="yp")
            for k in range(KT):
                nc.tensor.matmul(
                    ps[:],
                    w_sb[:, k, ts(m, P)],
                    xt_sb[:, n, k, :],
                    start=(k == 0),
                    stop=(k == KT - 1),
                )
            dst = tgt[:, m, ds(n * 512, 512)]
            if bias:
                nc.vector.tensor_scalar_add(dst, ps[:], bq2_sb[:, m : m + 1])
            else:
                nc.vector.tensor_copy(dst, ps[:])

        def projK(n):
            for m in range(2):
                proj_qk(n, m, kt_sb, wk_sb, False)

        def projQ(n, ms=(0, 1)):
            for m in ms:
                proj_qk(n, m, qt_sb, wq_sb, True)

        def projV(n, halves=(0, 1)):
            for half in halves:
                ps = psY.tile([P, 512], F32, name=f"ps_v{n}_{half}", tag="yp")
                for si2 in range(2):
                    sub = half * 2 + si2
                    for k in range(KT):
                        nc.tensor.matmul(
                            ps[:, ts(si2, 256)],
                            xt_sb[:, n, k, ts(sub, P)],
                            wv_sb[:, k, :],
                            start=(k == 0),
                            stop=(k == KT - 1),
                        )
                for si2 in range(2):
                    s = n * 4 + half * 2 + si2
                    dst = vaug_sb[:, s, 0 : HL * (DK + 1)].rearrange(
                        "p (h c) -> p h c", c=DK + 1
                    )[:, :, 0:DK]
                    src = ps[:, ds(si2 * 256, 256)].rearrange(
                        "p (h c) -> p h c", c=DK
                    )
                    nc.vector.tensor_copy(dst, src)

        # ---------------- attention ----------------
        y_r = y.rearrange("(t p) n -> t p n", p=P)

        def sc_block(qc, m, kb):
            """Score pair matmuls (concurrent K=64 streams), exp with pad
            bias, DVE causal mask on the diagonal block."""
            dd = kb - 4 * qc
            qoff = max(0, dd) * P
            w = 512 - qoff
            ps = psA.tile([P, 1024], F32, name=f"ps_a{qc}_{m}_{kb}", tag="ps")
            for hh in range(2):
                r0 = hh * DK
                nc.tensor.matmul(
                    ps[:, hh * 512 + qoff : (hh + 1) * 512],
                    kt_sb[r0 : r0 + DK, m, ds(kb * P, P)],
                    qt_sb[r0 : r0 + DK, m, ds(qc * 512 + qoff, w)],
                    start=True,
                    stop=True,
                )
            pt = pt_pool.tile([P, 1024], BF, name=f"pt{qc}_{m}_{kb}", tag="pt")
            ps3 = ps[:].rearrange("p (h q) -> p h q", h=2)[:, :, qoff:]
            pt3 = pt[:].rearrange("p (h q) -> p h q", h=2)[:, :, qoff:]
            nc.scalar.activation(
                pt3, ps3, AF.Exp, bias=padb_sb[:, kb : kb + 1], scale=0.125
            )
            if dd >= 0:
                ptd = pt[:].rearrange("p (h q) -> p h q", h=2)[:, :, qoff : qoff + P]
                trid = tri2_sb[:].rearrange("p (h q) -> p h q", h=2)
                nc.vector.tensor_mul(ptd, ptd, trid)
            return pt, qoff

        def pv_block(qc, m, kb, pvs, pt, qoff, nkb):
            for hh in range(2):
                h = 2 * m + hh
                nc.tensor.matmul(
                    pvs[hh][:, qoff:],
                    vaug_sb[:, kb, ds(h * (DK + 1), P)],
                    pt[:, hh * 512 + qoff : (hh + 1) * 512],
                    start=(kb == 0),
                    stop=(kb == nkb - 1),
                )

        def attn_phase(qc, m, fillers):
            """Emit one (qc, m) attention half-phase: scores run two blocks
            ahead of PV (psA ring depth), fillers[kb] callables are emitted
            at the top of iteration kb to keep the PE fed while the ACT
            drains the exp backlog. Late phases put the craw copies on the
            by-then idle ACT engine."""
            nkb = 4 * qc + 4
            pvs = [
                psB.tile([P, 512], F32, name=f"ctx{qc}_{m}_{i}", tag="ctx")
                for i in range(2)
            ]
            pend = {}
            pend[0] = sc_block(qc, m, 0)
            if nkb > 1:
                pend[1] = sc_block(qc, m, 1)
            for kb in range(nkb):
                for f in fillers.get(kb, ()):
                    f()
                if kb + 2 < nkb:
                    pend[kb + 2] = sc_block(qc, m, kb + 2)
                pt, qoff = pend.pop(kb)
                pv_block(qc, m, kb, pvs, pt, qoff, nkb)
            craws = []
            for hh in range(2):
                h = 2 * m + hh
                craw = rrp.tile(
                    [DK + 1, 512], BF, name=f"craw{qc}_{h}", tag="craw", bufs=5
                )
                if qc == 1:
                    nc.scalar.activation(craw[:], pvs[hh][0 : DK + 1, :], AF.Copy)
                else:
                    nc.vector.tensor_copy(craw[:], pvs[hh][0 : DK + 1, :])
                craws.append(craw)
            return craws

        def start_norm(qc, m, craws):
            """Kick off the reciprocal-broadcast chain (gpsimd DMAs + one DVE
            reciprocal); the multiplies are deferred to finish_norm one
            half-chunk later so no engine FIFO ever blocks on this chain."""
            den2 = rrp.tile([2, 512], F32, name=f"den2_{qc}_{m}", tag="den2", bufs=2)
            for hh in range(2):
                # bf16->f32 casting DMA: gpsimd only
                nc.gpsimd.dma_start(den2[hh : hh + 1, :], craws[hh][DK : DK + 1, :])
            rcp2 = rrp.tile([2, 512], F32, name=f"rcp2_{qc}_{m}", tag="rcp2", bufs=2)
            nc.vector.reciprocal_approx_fast(rcp2[:], den2[:])
            base = (qc * 2 + m) * 2
            nc.sync.dma_start(ins["rcp_dram"][base : base + 2, :], rcp2[:])
            rbs = []
            for hh in range(2):
                rb = rrp.tile(
                    [DK, 512], BF, name=f"rb{qc}_{m}{hh}", tag="rb", bufs=4
                )
                nc.gpsimd.dma_start(
                    rb[:],
                    ins["rcp_dram"][base + hh : base + hh + 1, :].to_broadcast(
                        [DK, 512]
                    ),
                )
                rbs.append(rb)
            return (qc, m, craws, rbs)

        def finish_norm(st):
            qc, m, craws, rbs = st
            ctx_pair = ctx_sets[qc % 2][m]
            tmp1 = rrp.tile([DK, 512], BF, name=f"tmp1_{qc}_{m}", tag="tmp1", bufs=2)
            nc.vector.tensor_mul(tmp1[:], craws[1][0:DK, :], rbs[1][:])
            nc.sync.dma_start(ctx_pair[DK:P, :], tmp1[:])
            nc.vector.tensor_mul(ctx_pair[0:DK, :], craws[0][0:DK, :], rbs[0][:])

        def fast_start(qc, m, craws):
            """Tail path, step 1: extract both denominators into one
            single-partition row (no partition bounce needed later); a couple
            of PE bridge matmuls anchored on the craw tiles keep the clock
            warm while the chain resolves."""
            den2 = rrp.tile([1, 1024], F32, name=f"den2f_{qc}_{m}", tag="den2f")
            for hh in range(2):
                nc.gpsimd.dma_start(
                    den2[0:1, ds(hh * 512, 512)], craws[hh][DK : DK + 1, :]
                )
            bridge = psA.tile([P, 1024], F32, name="bridge", tag="ps")
            for i in range(2):
                nc.tensor.matmul(
                    bridge[:, 0:512],
                    craws[1][0:DK, 0:P],
                    craws[1][0:DK, :],
                    start=True,
                    stop=True,
                )
            return den2

        def fast_finish(qc, m, craws, den2):
            """Tail path, step 2: per-head reciprocal + bf16 cast on
            partition 0, K=1 matmul broadcasts, multiplies. Head 1 stays at
            partitions 0-63 (tmp1); the tail out-projection consumes it
            there via a K=64 chain link, so no shift DMA is needed."""
            ctx_pair = ctx_sets[qc % 2][m]
            tmp1 = rrp.tile([DK, 512], BF, name=f"tmp1_{qc}_{m}", tag="tmp1", bufs=2)
            rcpb = rrp.tile([1, 1024], BF, name=f"rcpbf_{qc}_{m}", tag="rcpbf")
            for hh in range(2):
                den = den2[0:1, ds(hh * 512, 512)]
                nc.vector.reciprocal_approx_fast(den, den)
                nc.vector.tensor_copy(rcpb[0:1, ds(hh * 512, 512)], den)
            rb_list = []
            for hh in range(2):
                rb_ps = psB.tile([DK, 512], F32, name=f"rbp{qc}_{m}{hh}", tag="ctx")
                nc.tensor.matmul(
                    rb_ps[:],
                    ones_bf[0:1, 0:DK],
                    rcpb[0:1, ds(hh * 512, 512)],
                    start=True,
                    stop=True,
                )
                rb_list.append(rb_ps)
            # head 0 first: it unblocks the tail out-projection's 2nd link
            nc.vector.tensor_mul(ctx_pair[0:DK, :], craws[0][0:DK, :], rb_list[0][:])
            nc.vector.tensor_mul(tmp1[:], craws[1][0:DK, :], rb_list[1][:])
            return tmp1

        def outproj(qc, sis=(0, 1, 2, 3), tail=False, acts=(), tmp1=None):
            """acts: which nch copies go to the ACT engine (idle late in the
            kernel) instead of DVE. y DMAs ride sync (nch 0) / gpsimd or
            scalar (nch 1) so no single queue serializes the drain."""
            for si in sis:
                s = qc * 4 + si
                yt = ysb.tile([P, 1024], BF, name=f"yt{s}", tag="yt")
                for nch in range(2):
                    yp = psY.tile([P, 512], F32, name=f"yp{s}_{nch}", tag="yp")
                    if tmp1 is not None:
                        # 3-link chain: pair m0 (K=128), then heads 2 and 3
                        # separately (K=64) with head 3 read from tmp1 in
                        # place — avoids the partition-shift DMA on the tail
                        nc.tensor.matmul(
                            yp[:],
                            ctx_sets[qc % 2][0][:, ts(si, P)],
                            wo_sb[:, 0, ds(nch * 512, 512)],
                            start=True,
                            stop=False,
                        )
                        nc.tensor.matmul(
                            yp[:],
                            ctx_sets[qc % 2][1][0:DK, ts(si, P)],
                            wo_sb[0:DK, 1, ds(nch * 512, 512)],
                            start=False,
                            stop=False,
                        )
                        nc.tensor.matmul(
                            yp[:],
                            tmp1[:, ts(si, P)],
                            wo3_sb[:, ds(nch * 512, 512)],
                            start=False,
                            stop=True,
                        )
                    else:
                        for m in range(2):
                            nc.tensor.matmul(
                                yp[:],
                                ctx_sets[qc % 2][m][:, ts(si, P)],
                                wo_sb[:, m, ds(nch * 512, 512)],
                                start=(m == 0),
                                stop=(m == 1),
                            )
                    if nch in acts or (tail and nch == 1):
                        nc.scalar.activation(yt[:, ts(nch, 512)], yp[:], AF.Copy)
                    else:
                        nc.vector.tensor_copy(yt[:, ts(nch, 512)], yp[:])
                    if tail and nch == 1:
                        eng = nc.scalar
                    elif nch == 1:
                        eng = nc.gpsimd
                    else:
                        eng = nc.sync
                    eng.dma_start(
                        y_r[s][:, ds(nch * 512, 512)], yt[:, ts(nch, 512)]
                    )

        # ---------------- reversed interleaved schedule ----------------
        # Emission order IS the per-engine execution order. The first score
        # block needs only kt0 (wk+xt0) and qt3 (wq+xt3), matching the DMA
        # priority; all other projections ride as fillers inside qc3/m0.
        # No PE warm-up matmuls: the HAM grants one long full-duty window
        # (~90-130us of sustained activity) and then duty-cycles to 50%, so
        # junk matmuls spend that budget and slow the tail.
        projK(0)
        projQ(3)

        pend_n = None
        craws = attn_phase(3, 0, {
            0: [lambda: projV(0, halves=(0,))],
            1: [lambda: projV(0, halves=(1,))],
            2: [lambda: projK(1)],
            4: [lambda: projV(1)],
            6: [lambda: projK(2)],
            8: [lambda: projV(2)],
            9: [lambda: projK(3)],
            12: [lambda: projV(3)],
        })
        pend_n = start_norm(3, 0, craws)

        # qc3, m1: Q projections for chunks 2 and 1 as filler
        craws = attn_phase(3, 1, {
            1: [lambda: projQ(2, ms=(0,))],
            3: [lambda: projQ(2, ms=(1,))],
            5: [lambda: projQ(1, ms=(0,))],
            7: [lambda: projQ(1, ms=(1,))],
            12: [lambda: finish_norm(pend_n)],
        })
        pend_n = start_norm(3, 1, craws)

        # qc2, m0: finish qc3 norm, Q chunk 0, outproj(3) starts
        craws = attn_phase(2, 0, {
            0: [lambda: projQ(0, ms=(0,))],
            2: [lambda: projQ(0, ms=(1,))],
            5: [lambda: finish_norm(pend_n)],
            10: [lambda: outproj(3, sis=(0,))],
        })
        pend_n = start_norm(2, 0, craws)

        # qc2, m1
        craws = attn_phase(2, 1, {
            1: [lambda: outproj(3, sis=(1,))],
            4: [lambda: outproj(3, sis=(2,))],
            8: [lambda: finish_norm(pend_n)],
        })
        pend_n = start_norm(2, 1, craws)

        # qc0, m0 (short phases sit mid-stream, cushioned by outproj(3)/(2))
        craws = attn_phase(0, 0, {
            0: [lambda: finish_norm(pend_n)],
            2: [lambda: outproj(2, sis=(0,), acts=(1,))],
        })
        pend_n = start_norm(0, 0, craws)

        # qc0, m1: ALL of outproj(2) must precede finish_norm(0,0), which
        # overwrites the shared ctx slot (qc2 and qc0 both map to slot 0)
        craws = attn_phase(0, 1, {
            0: [lambda: outproj(2, sis=(1,), acts=(1,))],
            1: [lambda: outproj(2, sis=(2,), acts=(1,))],
            2: [lambda: outproj(2, sis=(3,), acts=(1,))],
            3: [lambda: finish_norm(pend_n)],
        })
        pend_n = start_norm(0, 1, craws)

        # qc1, m0: outproj(3,si3) before finish_norm(1,0) overwrites slot 1
        craws = attn_phase(1, 0, {
            1: [lambda: finish_norm(pend_n)],
            2: [lambda: outproj(3, sis=(3,))],
            4: [lambda: outproj(0, sis=(0,), acts=(1,))],
            6: [lambda: outproj(0, sis=(1,), acts=(1,))],
        })
        pend_n = start_norm(1, 0, craws)

        # qc1, m1 (tail): fast norm path, outproj(0) tail half bridges the
        # reciprocal chain latency
        craws = attn_phase(1, 1, {
            1: [lambda: outproj(0, sis=(2,), acts=(0, 1))],
            3: [lambda: outproj(0, sis=(3,), acts=(0, 1))],
            6: [lambda: finish_norm(pend_n)],
        })
        den2 = fast_start(1, 1, craws)
        tail_tmp1 = fast_finish(1, 1, craws, den2)
        outproj(1, tail=True, acts=(0, 1), tmp1=tail_tmp1)


# ---------------- host side ----------------

def make_in_maps(x, padding_mask, Wq, bq, Wk, Wv, Wo):
    """Build the 8 per-core input dicts from full inputs."""
    from ml_dtypes import bfloat16 as np_bf16

    x = np.asarray(x, dtype=np.float32)
    pad = np.asarray(padding_mask)
    # 0/1 lower-allowed triangle (mask k > q within a diagonal 128-block),
    # duplicated for the two heads of a row-packed pair
    tri01 = (np.arange(P)[:, None] <= np.arange(P)[None, :]).astype(np.float32)
    tri2 = np.tile(tri01, (1, 2)).astype(np_bf16)

    def swz_w(W):
        # [D, 256] transposed weight -> [P, KT, 256] partition-major
        wt = np.asarray(W, np.float32).T  # [D, 256]
        return np.ascontiguousarray(
            wt.reshape(KT, P, 2 * P).transpose(1, 0, 2)
        ).astype(np_bf16)

    in_maps = []
    for c in range(N_CORES):
        b, g = divmod(c, 4)
        R = slice(g * 256, g * 256 + 256)
        padb = np.where(pad[b] != 0, np.float32(0), np.float32(PADNEG)).astype(
            np.float32
        ).reshape(ST, P).T.copy()
        bq2 = np.ascontiguousarray(
            np.asarray(bq, np.float32)[R].reshape(2, P).T
        )
        # x[b].T is [D, S] = [(kt p), (nqc s)] -> [P, NQC, KT, 512]
        xtp = np.ascontiguousarray(
            x[b].T.reshape(KT, P, NQC, 512).transpose(1, 2, 0, 3)
        ).astype(np_bf16)
        in_maps.append(
            {
                "xt": xtp,
                "wq": swz_w(np.asarray(Wq, np.float32)[R, :]),
                "wk": swz_w(np.asarray(Wk, np.float32)[R, :]),
                "wv": swz_w(np.asarray(Wv, np.float32)[R, :]),
                "wo": np.ascontiguousarray(
                    np.asarray(Wo, np.float32)[:, R].T
                ).astype(np_bf16),
                "bq2": bq2,
                "padb": padb,
                "tri2": tri2,
            }
        )
    return in_maps


def postprocess(partials, x, padding_mask, Wv, bv, Wo, bo):
    """Sum per-core partials, add folded bias, fix fully-masked rows."""
    x = np.asarray(x, np.float32)
    pad = np.asarray(padding_mask)
    Wv = np.asarray(Wv, np.float32)
    bv = np.asarray(bv, np.float32)
    Wo = np.asarray(Wo, np.float32)
    bo = np.asarray(bo, np.float32)
    B = x.shape[0]
    y = np.zeros((B, S, D), dtype=np.float32)
    for c in range(N_CORES):
        y[c // 4] += np.asarray(partials[c], dtype=np.float32)
    y += (Wo @ bv + bo)[None, None, :]
    # fully-masked rows (reference: uniform attention over all keys)
    for b in range(B):
        nz = np.flatnonzero(pad[b] != 0)
        q0 = int(nz[0]) if len(nz) else S
        if q0 > 0:
            ctx_u = x[b].mean(axis=0) @ Wv.T + bv
            y[b, :q0, :] = ctx_u @ Wo.T + bo
    return y


_NC_CACHE = {}


def _get_program():
    if "nc" not in _NC_CACHE:
        _NC_CACHE["nc"] = build_program()
    return _NC_CACHE["nc"]


def kernel(
    x, padding_mask, Wq, bq, Wk, bk, Wv, bv, Wo, bo
):
    from concourse.bass_utils import run_bass_kernel_spmd

    nc = _get_program()
    in_maps = make_in_maps(x, padding_mask, Wq, bq, Wk, Wv, Wo)
    res = run_bass_kernel_spmd(nc, in_maps, core_ids=list(range(N_CORES)))
    partials = [res.results[c]["y"] for c in range(N_CORES)]
    return postprocess(partials, x, padding_mask, Wv, bv, Wo, bo)


# revision 40
# speedup vs baseline: 1.0539x; 1.0539x over previous
"""Self-contained Trainium2 Bass kernel for causal multi-head attention.

Problem: B=2, S=2048, D=1024, H=16 heads (dk=64), fp32, causal + padding mask.
Sharding across 8 NeuronCores: core c -> batch c//4, head-group c%4 (4 heads).

Device-side design (v4 — reversed phase order):
  - All matmul operands bf16 (cast on host); fp32 PSUM accumulation.
  - qT/kT transposed [dk, S]; scores S_T[k, q] with the head pair
    row-packed into PE rows 0-63 / 64-127: the two K=64 matmuls issue
    back-to-back and co-execute on disjoint row groups (measured 0-25ns
    issue deltas), so a score block pair costs ~512 cols, not 1024.
  - Padding via the exp's per-partition bias (-1e9 on padded keys).
  - Causal: DVE multiply of the exp'd diagonal 128-block by a 0/1
    triangle (replaces the v3 PE triangle-add matmuls: the exp of an
    unmasked score is finite, then zeroed, so PV and the denominator
    stay exact).
  - q bias via DVE tensor_scalar_add during the PSUM->SBUF copy
    (replaces the v3 K=1 bias matmuls). k bias provably cancels in
    softmax; v/o biases fold on host.
  - Query chunks processed HIGH to LOW (qc3..qc0). The exp work per
    chunk is ~2.5x its PE score+PV work, so late chunks (16 kb blocks)
    would starve the PE and trip the HAM 50% activity throttle at the
    tail; reversed, qc3's exp backlog overlaps the Q/V projections and
    the tail phase is the 4-block qc0 with out-projections as filler.
  - Deferred PE work (Q-proj of chunks 0-2, out-projections lagging one
    phase) is interleaved between attention blocks at ~1-3us grain so
    the PE queue never blocks on the ACT exp backlog.
  - Normalization (unchanged from v3): ctx^T+den in PSUM rows 0-64,
    copied once to SBUF; reciprocal via DVE; DMA broadcast through a
    DRAM bounce; multiplies deferred one half-phase. Tail fast path
    keeps the last heads' denominator on one partition.
Fully-masked rows are overwritten on host with the uniform-attention
reference value.
"""

import numpy as np
from contextlib import ExitStack

import concourse.bass as bass
import concourse.bacc as bacc
import concourse.tile as tile
import concourse.mybir as mybir
from concourse.bass import ds, ts

F32 = mybir.dt.float32
FR = mybir.dt.float32r
BF = mybir.dt.bfloat16
AF = mybir.ActivationFunctionType

P = 128
S = 2048
D = 1024
HL = 4          # heads per core
DK = 64
KT = D // P     # 8 k-tiles over the model dim
ST = S // P     # 16 seq tiles
NQC = 4         # 512-wide query chunks
PADNEG = -1.0e9  # post-scale padding bias; exp(-1e9) = 0
N_CORES = 8
N_HEAD = 16


def build_program(num_devices=N_CORES):
    nc = bacc.Bacc(
        "TRN2",
        target_bir_lowering=False,
        debug=False,
        enable_asserts=True,
        num_devices=num_devices,
    )
    ins = {
        # all bulk inputs pre-swizzled on host to partition-major layouts so
        # each arrives in 1-2 large contiguous DMA descriptors (the input
        # stream is descriptor-issue-bound, ~600ns engine time per dma_start)
        "xt": nc.dram_tensor("xt", [P, NQC, KT, 512], BF, kind="ExternalInput").ap(),
        "wq": nc.dram_tensor("wq", [P, KT, 2 * P], BF, kind="ExternalInput").ap(),
        "wk": nc.dram_tensor("wk", [P, KT, 2 * P], BF, kind="ExternalInput").ap(),
        "wv": nc.dram_tensor("wv", [P, KT, 2 * P], BF, kind="ExternalInput").ap(),
        "wo": nc.dram_tensor("wo", [2 * P, D], BF, kind="ExternalInput").ap(),
        "bq2": nc.dram_tensor("bq2", [P, 2], F32, kind="ExternalInput").ap(),
        "padb": nc.dram_tensor("padb", [P, ST], F32, kind="ExternalInput").ap(),
        "tri2": nc.dram_tensor("tri2", [P, 2 * P], BF, kind="ExternalInput").ap(),
    }
    y = nc.dram_tensor("y", [S, D], BF, kind="ExternalOutput").ap()
    ins["rcp_dram"] = nc.dram_tensor("rcp_dram", [2 * NQC * 2, 512], F32).ap()

    with tile.TileContext(nc) as tc:
        _body(tc, y, ins)

    nc.compile()
    return nc


def _body(tc, y, ins):
    nc = tc.nc

    with ExitStack() as ctx:
        const = ctx.enter_context(tc.tile_pool(name="const", bufs=1))
        pt_pool = ctx.enter_context(tc.tile_pool(name="pt", bufs=4))
        rrp = ctx.enter_context(tc.tile_pool(name="rr", bufs=2))
        ysb = ctx.enter_context(tc.tile_pool(name="ysb", bufs=4))
        psA = ctx.enter_context(tc.tile_pool(name="psA", bufs=2, space="PSUM"))
        psB = ctx.enter_context(tc.tile_pool(name="psB", bufs=2, space="PSUM"))
        psY = ctx.enter_context(tc.tile_pool(name="psY", bufs=2, space="PSUM"))

        # warmup operand memset FIRST so the PE can start immediately
        ones_sb = const.tile([1, 512], FR)
        nc.vector.memset(ones_sb[:].bitcast(F32), 1.0)
        ones_bf = const.tile([1, 512], BF)
        nc.vector.memset(ones_bf[:], 1.0)

        # ---------------- input DMAs ----------------
        # priority order, emissions alternating across the two HWDGE
        # queues: tiny tensors, wk, xt chunk3 (qc3 needs kt3+qt3 first),
        # wq, wv, xt chunks 0,1,2, wo last (needed only at outproj(3)).
        bq2_sb = const.tile([P, 2], F32)
        nc.sync.dma_start(bq2_sb[:], ins["bq2"])
        padb_sb = const.tile([P, ST], F32)
        nc.sync.dma_start(padb_sb[:], ins["padb"])
        tri2_sb = const.tile([P, 2 * P], BF)
        nc.sync.dma_start(tri2_sb[:], ins["tri2"])

        xt_sb = const.tile([P, NQC, KT, 512], BF)
        wq_sb = const.tile([P, KT, 2 * P], BF)
        wk_sb = const.tile([P, KT, 2 * P], BF)
        wv_sb = const.tile([P, KT, 2 * P], BF)

        # each tensor split k-wise in halves across the two HWDGE queues;
        # priority: wk, xt chunk3 (qc3 runs first), wq, wv, xt chunks 0,1,2
        def half_dma(dst, src):
            nc.sync.dma_start(dst[:, 0 : KT // 2], src[:, 0 : KT // 2])
            nc.scalar.dma_start(dst[:, KT // 2 :], src[:, KT // 2 :])

        half_dma(wk_sb, ins["wk"])
        half_dma(xt_sb[:, 0], ins["xt"][:, 0])
        half_dma(wq_sb, ins["wq"])
        half_dma(xt_sb[:, 3], ins["xt"][:, 3])
        half_dma(wv_sb, ins["wv"])
        for n in (1, 2):
            half_dma(xt_sb[:, n], ins["xt"][:, n])

        # wo: bf16, head pair m stacked as 128 contraction rows
        wo_sb = const.tile([P, 2, D], BF)
        wo_r = ins["wo"].rearrange("(m p) n -> m p n", p=P)
        for m in range(2):
            nc.scalar.dma_start(wo_sb[:, m], wo_r[m])
        # head 3's rows again at partitions 0-63: the tail out-projection
        # consumes head 3's ctx in place (no partition shift on the tail)
        wo3_sb = const.tile([DK, D], BF)
        nc.scalar.dma_start(wo3_sb[:], wo_r[1][DK:P, :])

        qt_sb = const.tile([P, 2, S], BF)
        kt_sb = const.tile([P, 2, S], BF)
        # per head: 64 value cols + ones denominator col; padded so a
        # 128-wide stationary slice starting at h*65 stays in bounds (the
        # extra columns produce junk output rows 65-127, never read)
        VW = HL * (DK + 1) + DK - 1  # 323
        vaug_sb = const.tile([P, ST, VW], BF)
        den_cols = vaug_sb[:, :, 0 : HL * (DK + 1)].rearrange(
            "p s (h c) -> p s h c", c=DK + 1
        )[:, :, :, DK : DK + 1]
        nc.vector.memset(den_cols, 1.0)
        nc.vector.memset(vaug_sb[:, :, HL * (DK + 1) : VW], 0.0)

        # normalized context head pairs [128, 512] bf16: rows 0-63 head 2m,
        # rows 64-127 head 2m+1 (DMA-shifted in)
        ctx_sets = []
        for st in range(2):
            tiles = []
            for m in range(2):
                t = const.tile([P, 512], BF, name=f"ctxsb{st}_{m}", tag=f"ctxsb{st}_{m}")
                tiles.append(t)
            ctx_sets.append(tiles)

        # The PE comes up in the HAM's 50%-duty state; ~3.4us of sustained
        # matmul activity un-throttles it. These dep-free fp32 matmuls run
        # during the input-DMA wait so the first real matmul is full-rate.
        def warm(n):
            warm_ps = psY.tile([P, 512], F32, name="warm", tag="yp")
            for i in range(n):
                nc.tensor.matmul(
                    warm_ps[:], ones_sb[:, 0:P], ones_sb[:], start=True, stop=True
                )

        # ---------------- projections (m-granular, psY ring) ----------------
        def proj_qk(n, m, tgt, w_sb, bias):
            """One [128,512] chunk of the q or k projection (chunk n, half m)."""
            ps = psY.tile([P, 512], F32, name=f"ps_{('k','q')[bias]}{n}_{m}", tag="yp")
            for k in range(KT):
                nc.tensor.matmul(
                    ps[:],
                    w_sb[:, k, ts(m, P)],
                    xt_sb[:, n, k, :],
                    start=(k == 0),
                    stop=(k == KT - 1),
                )
            dst = tgt[:, m, ds(n * 512, 512)]
            if bias:
                nc.vector.tensor_scalar_add(dst, ps[:], bq2_sb[:, m : m + 1])
            else:
                nc.vector.tensor_copy(dst, ps[:])

        def projK(n):
            for m in range(2):
                proj_qk(n, m, kt_sb, wk_sb, False)

        def projQ(n, ms=(0, 1)):
            for m in ms:
                proj_qk(n, m, qt_sb, wq_sb, True)

        def projV(n, halves=(0, 1)):
            for half in halves:
                ps = psY.tile([P, 512], F32, name=f"ps_v{n}_{half}", tag="yp")
                for si2 in range(2):
                    sub = half * 2 + si2
                    for k in range(KT):
                        nc.tensor.matmul(
                            ps[:, ts(si2, 256)],
                            xt_sb[:, n, k, ts(sub, P)],
                            wv_sb[:, k, :],
                            start=(k == 0),
                            stop=(k == KT - 1),
                        )
                for si2 in range(2):
                    s = n * 4 + half * 2 + si2
                    dst = vaug_sb[:, s, 0 : HL * (DK + 1)].rearrange(
                        "p (h c) -> p h c", c=DK + 1
                    )[:, :, 0:DK]
                    src = ps[:, ds(si2 * 256, 256)].rearrange(
                        "p (h c) -> p h c", c=DK
                    )
                    nc.vector.tensor_copy(dst, src)

        # ---------------- attention ----------------
        y_r = y.rearrange("(t p) n -> t p n", p=P)

        def sc_block(qc, m, kb):
            """Score pair matmuls (concurrent K=64 streams), exp with pad
            bias, DVE causal mask on the diagonal block."""
            dd = kb - 4 * qc
            qoff = max(0, dd) * P
            w = 512 - qoff
            ps = psA.tile([P, 1024], F32, name=f"ps_a{qc}_{m}_{kb}", tag="ps")
            for hh in range(2):
                r0 = hh * DK
                nc.tensor.matmul(
                    ps[:, hh * 512 + qoff : (hh + 1) * 512],
                    kt_sb[r0 : r0 + DK, m, ds(kb * P, P)],
                    qt_sb[r0 : r0 + DK, m, ds(qc * 512 + qoff, w)],
                    start=True,
                    stop=True,
                )
            pt = pt_pool.tile([P, 1024], BF, name=f"pt{qc}_{m}_{kb}", tag="pt")
            ps3 = ps[:].rearrange("p (h q) -> p h q", h=2)[:, :, qoff:]
            pt3 = pt[:].rearrange("p (h q) -> p h q", h=2)[:, :, qoff:]
            nc.scalar.activation(
                pt3, ps3, AF.Exp, bias=padb_sb[:, kb : kb + 1], scale=0.125
            )
            if dd >= 0:
                ptd = pt[:].rearrange("p (h q) -> p h q", h=2)[:, :, qoff : qoff + P]
                trid = tri2_sb[:].rearrange("p (h q) -> p h q", h=2)
                nc.vector.tensor_mul(ptd, ptd, trid)
            return pt, qoff

        def pv_block(qc, m, kb, pvs, pt, qoff, nkb):
            for hh in range(2):
                h = 2 * m + hh
                nc.tensor.matmul(
                    pvs[hh][:, qoff:],
                    vaug_sb[:, kb, ds(h * (DK + 1), P)],
                    pt[:, hh * 512 + qoff : (hh + 1) * 512],
                    start=(kb == 0),
                    stop=(kb == nkb - 1),
                )

        def attn_phase(qc, m, fillers):
            """Emit one (qc, m) attention half-phase: scores run two blocks
            ahead of PV (psA ring depth), fillers[kb] callables are emitted
            at the top of iteration kb to keep the PE fed while the ACT
            drains the exp backlog. Late phases put the craw copies on the
            by-then idle ACT engine."""
            nkb = 4 * qc + 4
            pvs = [
                psB.tile([P, 512], F32, name=f"ctx{qc}_{m}_{i}", tag="ctx")
                for i in range(2)
            ]
            pend = {}
            pend[0] = sc_block(qc, m, 0)
            if nkb > 1:
                pend[1] = sc_block(qc, m, 1)
            for kb in range(nkb):
                for f in fillers.get(kb, ()):
                    f()
                if kb + 2 < nkb:
                    pend[kb + 2] = sc_block(qc, m, kb + 2)
                pt, qoff = pend.pop(kb)
                pv_block(qc, m, kb, pvs, pt, qoff, nkb)
            craws = []
            for hh in range(2):
                h = 2 * m + hh
                craw = rrp.tile(
                    [DK + 1, 512], BF, name=f"craw{qc}_{h}", tag="craw", bufs=5
                )
                if qc == 1:
                    nc.scalar.activation(craw[:], pvs[hh][0 : DK + 1, :], AF.Copy)
                else:
                    nc.vector.tensor_copy(craw[:], pvs[hh][0 : DK + 1, :])
                craws.append(craw)
            return craws

        def start_norm(qc, m, craws):
            """Kick off the reciprocal-broadcast chain (gpsimd DMAs + one DVE
            reciprocal); the multiplies are deferred to finish_norm one
            half-chunk later so no engine FIFO ever blocks on this chain."""
            den2 = rrp.tile([2, 512], F32, name=f"den2_{qc}_{m}", tag="den2", bufs=2)
            for hh in range(2):
                # bf16->f32 casting DMA: gpsimd only
                nc.gpsimd.dma_start(den2[hh : hh + 1, :], craws[hh][DK : DK + 1, :])
            rcp2 = rrp.tile([2, 512], F32, name=f"rcp2_{qc}_{m}", tag="rcp2", bufs=2)
            nc.vector.reciprocal_approx_fast(rcp2[:], den2[:])
            base = (qc * 2 + m) * 2
            nc.sync.dma_start(ins["rcp_dram"][base : base + 2, :], rcp2[:])
            rbs = []
            for hh in range(2):
                rb = rrp.tile(
                    [DK, 512], BF, name=f"rb{qc}_{m}{hh}", tag="rb", bufs=4
                )
                nc.gpsimd.dma_start(
                    rb[:],
                    ins["rcp_dram"][base + hh : base + hh + 1, :].to_broadcast(
                        [DK, 512]
                    ),
                )
                rbs.append(rb)
            return (qc, m, craws, rbs)

        def finish_norm(st):
            qc, m, craws, rbs = st
            ctx_pair = ctx_sets[qc % 2][m]
            tmp1 = rrp.tile([DK, 512], BF, name=f"tmp1_{qc}_{m}", tag="tmp1", bufs=2)
            nc.vector.tensor_mul(tmp1[:], craws[1][0:DK, :], rbs[1][:])
            nc.sync.dma_start(ctx_pair[DK:P, :], tmp1[:])
            nc.vector.tensor_mul(ctx_pair[0:DK, :], craws[0][0:DK, :], rbs[0][:])

        def fast_start(qc, m, craws):
            """Tail path, step 1: extract both denominators into one
            single-partition row (no partition bounce needed later); a couple
            of PE bridge matmuls anchored on the craw tiles keep the clock
            warm while the chain resolves."""
            den2 = rrp.tile([1, 1024], F32, name=f"den2f_{qc}_{m}", tag="den2f")
            for hh in range(2):
                nc.gpsimd.dma_start(
                    den2[0:1, ds(hh * 512, 512)], craws[hh][DK : DK + 1, :]
                )
            bridge = psA.tile([P, 1024], F32, name="bridge", tag="ps")
            for i in range(2):
                nc.tensor.matmul(
                    bridge[:, 0:512],
                    craws[1][0:DK, 0:P],
                    craws[1][0:DK, :],
                    start=True,
                    stop=True,
                )
            return den2

        def fast_finish(qc, m, craws, den2):
            """Tail path, step 2: per-head reciprocal + bf16 cast on
            partition 0, K=1 matmul broadcasts, multiplies. Head 1 stays at
            partitions 0-63 (tmp1); the tail out-projection consumes it
            there via a K=64 chain link, so no shift DMA is needed."""
            ctx_pair = ctx_sets[qc % 2][m]
            tmp1 = rrp.tile([DK, 512], BF, name=f"tmp1_{qc}_{m}", tag="tmp1", bufs=2)
            rcpb = rrp.tile([1, 1024], BF, name=f"rcpbf_{qc}_{m}", tag="rcpbf")
            for hh in range(2):
                den = den2[0:1, ds(hh * 512, 512)]
                nc.vector.reciprocal_approx_fast(den, den)
                nc.vector.tensor_copy(rcpb[0:1, ds(hh * 512, 512)], den)
            rb_list = []
            for hh in range(2):
                rb_ps = psB.tile([DK, 512], F32, name=f"rbp{qc}_{m}{hh}", tag="ctx")
                nc.tensor.matmul(
                    rb_ps[:],
                    ones_bf[0:1, 0:DK],
                    rcpb[0:1, ds(hh * 512, 512)],
                    start=True,
                    stop=True,
                )
                rb_list.append(rb_ps)
            # head 0 first: it unblocks the tail out-projection's 2nd link
            nc.vector.tensor_mul(ctx_pair[0:DK, :], craws[0][0:DK, :], rb_list[0][:])
            nc.vector.tensor_mul(tmp1[:], craws[1][0:DK, :], rb_list[1][:])
            return tmp1

        def outproj(qc, sis=(0, 1, 2, 3), tail=False, acts=(), tmp1=None):
            """acts: which nch copies go to the ACT engine (idle late in the
            kernel) instead of DVE. y DMAs ride sync (nch 0) / gpsimd or
            scalar (nch 1) so no single queue serializes the drain."""
            for si in sis:
                s = qc * 4 + si
                yt = ysb.tile([P, 1024], BF, name=f"yt{s}", tag="yt")
                for nch in range(2):
                    yp = psY.tile([P, 512], F32, name=f"yp{s}_{nch}", tag="yp")
                    if tmp1 is not None:
                        # 3-link chain: pair m0 (K=128), then heads 2 and 3
                        # separately (K=64) with head 3 read from tmp1 in
                        # place — avoids the partition-shift DMA on the tail
                        nc.tensor.matmul(
                            yp[:],
                            ctx_sets[qc % 2][0][:, ts(si, P)],
                            wo_sb[:, 0, ds(nch * 512, 512)],
                            start=True,
                            stop=False,
                        )
                        nc.tensor.matmul(
                            yp[:],
                            ctx_sets[qc % 2][1][0:DK, ts(si, P)],
                            wo_sb[0:DK, 1, ds(nch * 512, 512)],
                            start=False,
                            stop=False,
                        )
                        nc.tensor.matmul(
                            yp[:],
                            tmp1[:, ts(si, P)],
                            wo3_sb[:, ds(nch * 512, 512)],
                            start=False,
                            stop=True,
                        )
                    else:
                        for m in range(2):
                            nc.tensor.matmul(
                                yp[:],
                                ctx_sets[qc % 2][m][:, ts(si, P)],
                                wo_sb[:, m, ds(nch * 512, 512)],
                                start=(m == 0),
                                stop=(m == 1),
                            )
                    if nch in acts or (tail and nch == 1):
                        nc.scalar.activation(yt[:, ts(nch, 512)], yp[:], AF.Copy)
                    else:
                        nc.vector.tensor_copy(yt[:, ts(nch, 512)], yp[:])
                    if tail and nch == 1:
                        eng = nc.scalar
                    elif nch == 1:
                        eng = nc.gpsimd
                    else:
                        eng = nc.sync
                    eng.dma_start(
                        y_r[s][:, ds(nch * 512, 512)], yt[:, ts(nch, 512)]
                    )

        # ---------------- reversed interleaved schedule ----------------
        # Emission order IS the per-engine execution order. The first score
        # block needs only kt0 (wk+xt0) and qt3 (wq+xt3), matching the DMA
        # priority; all other projections ride as fillers inside qc3/m0.
        warm(9)
        projK(0)
        projQ(3)

        pend_n = None
        craws = attn_phase(3, 0, {
            0: [lambda: projV(0, halves=(0,))],
            1: [lambda: projV(0, halves=(1,))],
            2: [lambda: projK(1)],
            4: [lambda: projV(1)],
            6: [lambda: projK(2)],
            8: [lambda: projV(2)],
            9: [lambda: projK(3)],
            12: [lambda: projV(3)],
        })
        pend_n = start_norm(3, 0, craws)

        # qc3, m1: Q projections for chunks 2 and 1 as filler
        craws = attn_phase(3, 1, {
            1: [lambda: projQ(2, ms=(0,))],
            3: [lambda: projQ(2, ms=(1,))],
            5: [lambda: projQ(1, ms=(0,))],
            7: [lambda: projQ(1, ms=(1,))],
            12: [lambda: finish_norm(pend_n)],
        })
        pend_n = start_norm(3, 1, craws)

        # qc2, m0: finish qc3 norm, Q chunk 0, outproj(3) starts
        craws = attn_phase(2, 0, {
            0: [lambda: projQ(0, ms=(0,))],
            2: [lambda: projQ(0, ms=(1,))],
            5: [lambda: finish_norm(pend_n)],
            10: [lambda: outproj(3, sis=(0,))],
        })
        pend_n = start_norm(2, 0, craws)

        # qc2, m1
        craws = attn_phase(2, 1, {
            1: [lambda: outproj(3, sis=(1,))],
            4: [lambda: outproj(3, sis=(2,))],
            8: [lambda: finish_norm(pend_n)],
        })
        pend_n = start_norm(2, 1, craws)

        # qc0, m0 (short phases sit mid-stream; the qc2/qc0 finish chains
        # are deferred into the roomy (1,0) phase so their DMA-broadcast
        # latency never blocks these 4-block phases)
        craws = attn_phase(0, 0, {
            1: [lambda: outproj(3, sis=(3,))],
        })
        pend_00 = start_norm(0, 0, craws)

        # qc0, m1: ALL of outproj(2) must precede finish_norm(0,0), which
        # overwrites the shared ctx slot (qc2 and qc0 both map to slot 0)
        craws = attn_phase(0, 1, {
            0: [lambda: finish_norm(pend_n)],
            1: [lambda: outproj(2, sis=(0,), acts=(1,)),
                lambda: outproj(2, sis=(1,), acts=(1,))],
            2: [lambda: outproj(2, sis=(2,), acts=(1,))],
        })
        pend_01 = start_norm(0, 1, craws)

        # qc1, m0: qc0's norm chains resolve here with full slack; all of
        # outproj(0) lands in this phase to keep the tail PE-light
        craws = attn_phase(1, 0, {
            0: [lambda: outproj(2, sis=(3,), acts=(1,))],
            1: [lambda: finish_norm(pend_00)],
            3: [lambda: finish_norm(pend_01)],
            4: [lambda: outproj(0, sis=(0,), acts=(1,))],
            5: [lambda: outproj(0, sis=(1,), acts=(1,))],
            6: [lambda: outproj(0, sis=(2,), acts=(1,))],
            7: [lambda: outproj(0, sis=(3,), acts=(1,))],
        })
        pend_n = start_norm(1, 0, craws)

        # qc1, m1 (tail): fast norm path; outproj(1) is the only PE work
        # left after the last attention block
        craws = attn_phase(1, 1, {
            4: [lambda: finish_norm(pend_n)],
        })
        den2 = fast_start(1, 1, craws)
        tail_tmp1 = fast_finish(1, 1, craws, den2)
        outproj(1, tail=True, acts=(0, 1), tmp1=tail_tmp1)


# ---------------- host side ----------------

def make_in_maps(x, padding_mask, Wq, bq, Wk, Wv, Wo):
    """Build the 8 per-core input dicts from full inputs."""
    from ml_dtypes import bfloat16 as np_bf16

    x = np.asarray(x, dtype=np.float32)
    pad = np.asarray(padding_mask)
    # 0/1 lower-allowed triangle (mask k > q within a diagonal 128-block),
    # duplicated for the two heads of a row-packed pair
    tri01 = (np.arange(P)[:, None] <= np.arange(P)[None, :]).astype(np.float32)
    tri2 = np.tile(tri01, (1, 2)).astype(np_bf16)

    def swz_w(W):
        # [D, 256] transposed weight -> [P, KT, 256] partition-major
        wt = np.asarray(W, np.float32).T  # [D, 256]
        return np.ascontiguousarray(
            wt.reshape(KT, P, 2 * P).transpose(1, 0, 2)
        ).astype(np_bf16)

    in_maps = []
    for c in range(N_CORES):
        b, g = divmod(c, 4)
        R = slice(g * 256, g * 256 + 256)
        padb = np.where(pad[b] != 0, np.float32(0), np.float32(PADNEG)).astype(
            np.float32
        ).reshape(ST, P).T.copy()
        bq2 = np.ascontiguousarray(
            np.asarray(bq, np.float32)[R].reshape(2, P).T
        )
        # x[b].T is [D, S] = [(kt p), (nqc s)] -> [P, NQC, KT, 512]
        xtp = np.ascontiguousarray(
            x[b].T.reshape(KT, P, NQC, 512).transpose(1, 2, 0, 3)
        ).astype(np_bf16)
        in_maps.append(
            {
                "xt": xtp,
                "wq": swz_w(np.asarray(Wq, np.float32)[R, :]),
                "wk": swz_w(np.asarray(Wk, np.float32)[R, :]),
                "wv": swz_w(np.asarray(Wv, np.float32)[R, :]),
                "wo": np.ascontiguousarray(
                    np.asarray(Wo, np.float32)[:, R].T
                ).astype(np_bf16),
                "bq2": bq2,
                "padb": padb,
                "tri2": tri2,
            }
        )
    return in_maps


def postprocess(partials, x, padding_mask, Wv, bv, Wo, bo):
    """Sum per-core partials, add folded bias, fix fully-masked rows."""
    x = np.asarray(x, np.float32)
    pad = np.asarray(padding_mask)
    Wv = np.asarray(Wv, np.float32)
    bv = np.asarray(bv, np.float32)
    Wo = np.asarray(Wo, np.float32)
    bo = np.asarray(bo, np.float32)
    B = x.shape[0]
    y = np.zeros((B, S, D), dtype=np.float32)
    for c in range(N_CORES):
        y[c // 4] += np.asarray(partials[c], dtype=np.float32)
    y += (Wo @ bv + bo)[None, None, :]
    # fully-masked rows (reference: uniform attention over all keys)
    for b in range(B):
        nz = np.flatnonzero(pad[b] != 0)
        q0 = int(nz[0]) if len(nz) else S
        if q0 > 0:
            ctx_u = x[b].mean(axis=0) @ Wv.T + bv
            y[b, :q0, :] = ctx_u @ Wo.T + bo
    return y


_NC_CACHE = {}


def _get_program():
    if "nc" not in _NC_CACHE:
        _NC_CACHE["nc"] = build_program()
    return _NC_CACHE["nc"]


def kernel(
    x, padding_mask, Wq, bq, Wk, bk, Wv, bv, Wo, bo
):
    from concourse.bass_utils import run_bass_kernel_spmd

    nc = _get_program()
    in_maps = make_in_maps(x, padding_mask, Wq, bq, Wk, Wv, Wo)
    res = run_bass_kernel_spmd(nc, in_maps, core_ids=list(range(N_CORES)))
    partials = [res.results[c]["y"] for c in range(N_CORES)]
    return postprocess(partials, x, padding_mask, Wv, bv, Wo, bo)
